# revision 2
# baseline (speedup 1.0000x reference)
"""AttnBlock3d (GroupNorm -> QKV -> softmax attention -> proj -> residual) on 8 trn2 cores.

Sharding: 8 shards = batch (2) x query-chunk (4 x 1024 tokens). Each core receives the
full batch slice (for GN stats and K/V) plus its query chunk; per-core difference is
entirely in the input data, so one SPMD NEFF runs on all 8 cores with no collectives.
Host gathers the per-core [C, 1024] outputs back into [2, C, 16, 16, 16].

v2 structure (vs the 83us baseline):
- Startup: xb chunks stream on the sync HWDGE queue (staggered arrival feeds the
  GN stats as they land), weights on the scalar HWDGE queue, gpsimd does no DMA.
  xq (f32 residual) is only needed at the end and streams after xb.
- Host rotates the xb chunk axis per core so chunk 0 is always the core's query
  chunk (key order is permutation-invariant through S->P->V->O); Hq is then built
  from the fp8 x_pk directly, taking xq off the critical path.
- K and Q are never materialized: S^T = x^T G'' with G'' folded from the GN affine,
  bq, and Wq^T Wk (as in the baseline).
- S loop is evac-bound (ACT exact exp / DVE Schraudolph split + single-op V evacs,
  balanced by measured rates); two of the eight O accumulation chains ride the
  S loop's spare PE cycles in the 2 leftover PSUM banks.
- O^T is produced by HWDGE DMA-transpose (bf16) instead of PE transposes; Wp stays
  bf16 (no fp8 2^13 scale trick) and Wp@cv folds into the residual prep, so the
  entire ot-evac ACT pass disappears.
"""

import ml_dtypes
import numpy as np

import concourse.bacc as bacc
import concourse.mybir as mybir
import concourse.tile as tile
from concourse.bass_utils import run_bass_kernel_spmd

B = 2
C = 256
G = 32
N = 4096          # D*H*W tokens per batch
NQ = 1024         # query chunk per core
EPS = 1e-5
SCALE = 1.0 / 16.0  # C ** -0.5
F32 = mybir.dt.float32
BF16 = mybir.dt.bfloat16
FP8 = mybir.dt.float8e4
U8 = mybir.dt.uint8
I32 = mybir.dt.int32
NT = N // 128     # 32 key tiles
NJ = NT // 2      # 16 key-pair blocks
NQT = NQ // 128   # 8 query tiles per core
GH = G // 2
WARMUP_MMS = 14

# Schraudolph fast-exp: exp(x) ~= bitcast_fp8e4(uint8(x * 8*log2(e) + 55.63))
EXP_A = 8.0 * 1.4426950408889634
EXP_B = 56.0 - 0.37
LOGIT_BIAS = -3.0  # softmax shift: exp(s/16 - 3) keeps fp8/u8 in range

# j indices whose V evac goes to ACT (9 of 16; DVE takes the rest)
ACT_V_JS = {0, 2, 4, 5, 8, 10, 12, 14, 6}

# vecs layout along the free dim: gamma, beta, bq, bv, bp
VG, VB, VBQ, VBV, VBP = range(5)


def build_nc():
    nc = bacc.Bacc("TRN2", target_bir_lowering=False, debug=False, num_devices=8)

    # x channel-packed fp8: [chunk 4, 128, (s=2, n=1024)]; chunk 0 = query chunk
    xb = nc.dram_tensor("xb", [4, 128, 2048], FP8, kind="ExternalInput").ap()
    xq = nc.dram_tensor("xq", [C, NQ], F32, kind="ExternalInput").ap()
    wqnt = nc.dram_tensor("wqnt", [C, C], BF16, kind="ExternalInput").ap()
    wknt = nc.dram_tensor("wknt", [C, C], BF16, kind="ExternalInput").ap()
    wvT = nc.dram_tensor("wvT", [C, C], F32, kind="ExternalInput").ap()
    wpT = nc.dram_tensor("wpT", [2, 128, C], BF16, kind="ExternalInput").ap()
    vecs = nc.dram_tensor("vecs", [128, 2, 5], F32, kind="ExternalInput").ap()
    ig = nc.dram_tensor("ig", [2, 128, GH], F32, kind="ExternalInput").ap()
    igt = nc.dram_tensor("igt", [2, GH, 128], F32, kind="ExternalInput").ap()
    y = nc.dram_tensor("y", [2, 2, 128, 512], BF16, kind="ExternalOutput").ap()

    from concourse.masks import make_identity

    with tile.TileContext(nc) as tc:
        with (
            tc.tile_pool(name="consts", bufs=1) as consts,
            tc.tile_pool(name="small", bufs=1) as small,
            tc.tile_pool(name="kqv", bufs=1) as kqv,
            tc.tile_pool(name="attn", bufs=1) as attn,
        ):
            # ---- input DMAs: xb chunks sequentially on sync (stats eat them in
            # arrival order), then xq; weights on scalar ----
            x_pk = kqv.tile([128, 4, 2, 1024], FP8, tag="xpk", name="xpk")
            for c in range(4):
                nc.sync.dma_start(out=x_pk[:, c], in_=xb[c])
            xq_f = [kqv.tile([128, NQ], F32, tag=f"xqf{t}", name=f"xqf{t}") for t in range(2)]
            for t in range(2):
                nc.sync.dma_start(out=xq_f[t], in_=xq[t * 128:(t + 1) * 128, :])

            wq_nt = [consts.tile([128, C], BF16, tag=f"wqnt{t}", name=f"wqnt{t}") for t in range(2)]
            wk_nt = [consts.tile([128, C], BF16, tag=f"wknt{t}", name=f"wknt{t}") for t in range(2)]
            for t in range(2):
                nc.scalar.dma_start(out=wq_nt[t], in_=wqnt[t * 128:(t + 1) * 128, :])
            for t in range(2):
                nc.scalar.dma_start(out=wk_nt[t], in_=wknt[t * 128:(t + 1) * 128, :])
            vecs2_sb = consts.tile([128, 2, 5], F32, tag="vecs2", name="vecs2")
            vecs_t = [vecs2_sb[:, t, :] for t in range(2)]
            nc.scalar.dma_start(out=vecs2_sb, in_=vecs)
            ig_t = [consts.tile([128, GH], F32, tag=f"ig{t}", name=f"ig{t}") for t in range(2)]
            igt_sb = [consts.tile([GH, 128], F32, tag=f"igt{t}", name=f"igt{t}")
                      for t in range(2)]
            for t in range(2):
                nc.scalar.dma_start(out=ig_t[t], in_=ig[t])
            for t in range(2):
                nc.scalar.dma_start(out=igt_sb[t], in_=igt[t])
            wraw_v = []
            for t in range(2):
                wt = consts.tile([128, C], F32, tag=f"wv{t}", name=f"wv{t}")
                nc.scalar.dma_start(out=wt, in_=wvT[t * 128:(t + 1) * 128, :])
                wraw_v.append(wt)
            wpT_sb = consts.tile([128, 2, C], BF16, tag="wpT", name="wpT")
            nc.scalar.dma_start(out=wpT_sb, in_=wpT.rearrange("t p c -> p t c"))

            # small SBUF constants on gpsimd (no DMA there, just compute)
            ident = consts.tile([128, 128], BF16, tag="ident", name="ident")
            warm_rhs = consts.tile([128, 512], BF16, tag="warm", name="warm")
            make_identity(nc, ident)
            nc.gpsimd.memset(warm_rhs, 0.25)
            ebias = small.tile([128, 1], F32, tag="ebias", name="ebias")
            nc.gpsimd.memset(ebias, LOGIT_BIAS)

            g_pk = kqv.tile([128, 2, NQ], FP8, tag="gpk", name="gpk")
            hq_bf = [kqv.tile([128, NQ], BF16, tag=f"hq{t}", name=f"hq{t}") for t in range(2)]
            m2_sb = [kqv.tile([128, C], BF16, tag=f"m2{t}", name=f"m2{t}") for t in range(2)]
            wv_pk = consts.tile([128, 2, C], FP8, tag="wvpk", name="wvpk")
            vt1 = [kqv.tile([128, 2, C + 16], FP8, tag=f"vt{j}", name=f"vt{j}")
                   for j in range(NJ)]
            pt = [attn.tile([128, 2, NQ], FP8, tag=f"pt{j}", name=f"pt{j}")
                  for j in range(NJ)]
            for j in range(NJ):
                nc.gpsimd.memset(vt1[j][:, :, C:C + 1], 1.0)

            a2 = small.tile([128, 2], F32, tag="a2", name="a2")
            b2 = small.tile([128, 2], F32, tag="b2", name="b2")
            w22 = small.tile([128, 2], F32, tag="w22", name="w22")
            a_t = [a2[:, t:t + 1] for t in range(2)]
            b_t = [b2[:, t:t + 1] for t in range(2)]
            w2 = [w22[:, t:t + 1] for t in range(2)]
            cv = [small.tile([128, 1], F32, tag=f"cv{m}", name=f"cv{m}") for m in range(2)]
            cvbf = small.tile([128, 2], BF16, tag="cvbf", name="cvbf")
            bpv2 = small.tile([128, 2], F32, tag="bpv2", name="bpv2")
            bq2 = small.tile([128, 2], BF16, tag="bq2", name="bq2")
            pdum = small.tile([32, 1], F32, tag="pdum", name="pdum")

            def xsg(t, sg):  # [128, 512] subgroup sg of c-tile t, chunk-major
                return x_pk[:, sg // 2, t, (sg % 2) * 512:(sg % 2 + 1) * 512]

            with tc.tile_pool(name="pspre", bufs=1, space="PSUM") as pspre:
                # PE warmup while DMAs stream; preload the exp ACT table set.
                wp_ps = pspre.tile([128, 512], F32, tag="warmps", name="warmps")
                for _ in range(WARMUP_MMS):
                    nc.tensor.matmul(wp_ps, lhsT=ident, rhs=warm_rhs, start=True, stop=True)
                nc.scalar.activation(out=pdum, in_=ident[0:32, 0:1],
                                     func=mybir.ActivationFunctionType.Exp, scale=1.0)

                # M2 = (Wq^T Wk) tiles: m2_sb[cs][p, f] = Mk[f, cs*128+p]
                for cs in range(2):
                    m2_ps = pspre.tile([128, C], F32, tag="m2ps", name="m2ps", bufs=2)
                    for ot in range(2):
                        nc.tensor.matmul(m2_ps, lhsT=wq_nt[ot][:, cs * 128:(cs + 1) * 128],
                                         rhs=wk_nt[ot], start=(ot == 0), stop=(ot == 1))
                    if cs == 0:
                        nc.scalar.copy(out=m2_sb[cs], in_=m2_ps)
                    else:
                        nc.vector.tensor_copy(out=m2_sb[cs], in_=m2_ps)
                # w~ = Wk^T bq (per c-tile), later scaled by a into w2
                nc.vector.tensor_copy(out=bq2, in_=vecs2_sb[:, :, VBQ])
                wt_ps2 = pspre.tile([128, 2], F32, tag="wtps", name="wtps")
                wt_ps = [wt_ps2[:, ct:ct + 1] for ct in range(2)]
                for ct in range(2):
                    for ot in range(2):
                        nc.tensor.matmul(wt_ps[ct],
                                         lhsT=wk_nt[ot][:, ct * 128:(ct + 1) * 128],
                                         rhs=bq_half(bq2, ot), start=(ot == 0), stop=(ot == 1))

                # keep the PE busy across the stats window (HAM stays warm)
                for _ in range(4):
                    nc.tensor.matmul(wp_ps, lhsT=ident, rhs=warm_rhs, start=True, stop=True)

                # ---- GN stats in chunk-arrival order ----
                # c-tile 0 + chunk 3 of c-tile 1: DVE bn_stats (10 subgroups);
                # chunks 0-2 of c-tile 1: ACT Identity/Square accumulations.
                st = [small.tile([128, 2], F32, tag=f"st{t}", name=f"st{t}") for t in range(2)]

                stats8a = small.tile([128, 8, 6], F32, tag="stats8a", name="stats8a")
                mva = small.tile([128, 2], F32, tag="mva", name="mva")
                stats2 = small.tile([128, 2, 6], F32, tag="stats2", name="stats2")
                mv = small.tile([128, 2], F32, tag="mv", name="mv")
                acc = small.tile([128, 3, 2], F32, tag="acc", name="acc")

                for i, sg in enumerate(range(8)):
                    nc.vector.bn_stats(out=stats8a[:, i, :], in_=xsg(0, sg))
                for i in range(2):
                    nc.vector.bn_stats(out=stats2[:, i, :], in_=xsg(1, 6 + i))
                for i in range(3):
                    rng = x_pk[:, i, 1, :]
                    junk = small.tile([128, 1024], BF16, tag="junk", name="junk", bufs=2)
                    nc.scalar.activation(out=junk, in_=rng,
                                         func=mybir.ActivationFunctionType.Identity,
                                         accum_out=acc[:, i, 0:1])
                    nc.scalar.activation(out=junk, in_=rng,
                                         func=mybir.ActivationFunctionType.Square,
                                         accum_out=acc[:, i, 1:2])
                nc.vector.bn_aggr(out=mva, in_=stats8a)
                nc.vector.tensor_copy(out=st[0][:, 0:1], in_=mva[:, 0:1])
                nc.vector.tensor_mul(out=st[0][:, 1:2], in0=mva[:, 0:1], in1=mva[:, 0:1])
                nc.vector.tensor_add(out=st[0][:, 1:2], in0=st[0][:, 1:2], in1=mva[:, 1:2])

                nc.vector.bn_aggr(out=mv, in_=stats2)
                n_dve = float(2 * 1024)
                n_tot = float(N)
                sums = small.tile([128, 2], F32, tag="sums", name="sums")
                nc.vector.tensor_tensor(out=sums, in0=acc[:, 0, :],
                                        in1=acc[:, 1, :], op=mybir.AluOpType.add)
                nc.vector.tensor_tensor(out=sums, in0=sums,
                                        in1=acc[:, 2, :], op=mybir.AluOpType.add)
                # E[x] = (mean_dve*n_dve + sum_act)/N
                nc.vector.scalar_tensor_tensor(out=st[1][:, 0:1], in0=mv[:, 0:1],
                                               scalar=n_dve, in1=sums[:, 0:1],
                                               op0=mybir.AluOpType.mult,
                                               op1=mybir.AluOpType.add)
                nc.vector.tensor_scalar_mul(out=st[1][:, 0:1], in0=st[1][:, 0:1],
                                            scalar1=1.0 / n_tot)
                # E[x^2] = ((var+mean^2)*n_dve + sumsq_act)/N
                nc.vector.tensor_mul(out=st[1][:, 1:2], in0=mv[:, 0:1], in1=mv[:, 0:1])
                nc.vector.tensor_add(out=st[1][:, 1:2], in0=st[1][:, 1:2], in1=mv[:, 1:2])
                nc.vector.scalar_tensor_tensor(out=st[1][:, 1:2], in0=st[1][:, 1:2],
                                               scalar=n_dve, in1=sums[:, 1:2],
                                               op0=mybir.AluOpType.mult,
                                               op1=mybir.AluOpType.add)
                nc.vector.tensor_scalar_mul(out=st[1][:, 1:2], in0=st[1][:, 1:2],
                                            scalar1=1.0 / n_tot)

                grs_t = []
                for t in range(2):
                    # per-tile group stats -> rsqrt chain
                    ps_g = pspre.tile([GH, 2], F32, tag="gst", name="gst", bufs=2)
                    nc.tensor.matmul(ps_g, lhsT=ig_t[t],
                                     rhs=st[t], start=True, stop=True)
                    tg1 = small.tile([GH, 1], F32, tag=f"tg1{t}", name=f"tg1{t}")
                    tg2 = small.tile([GH, 1], F32, tag=f"tg2{t}", name=f"tg2{t}")
                    tg3 = small.tile([GH, 1], F32, tag=f"tg3{t}", name=f"tg3{t}")
                    tg4 = small.tile([GH, 1], F32, tag=f"tg4{t}", name=f"tg4{t}")
                    grs = small.tile([GH, 2], F32, tag=f"grs{t}", name=f"grs{t}")
                    nc.vector.tensor_copy(out=grs[:, 0:1], in_=ps_g[:, 0:1])
                    nc.vector.tensor_mul(out=tg1, in0=grs[:, 0:1], in1=grs[:, 0:1])
                    nc.vector.tensor_tensor(out=tg1, in0=ps_g[:, 1:2], in1=tg1,
                                            op=mybir.AluOpType.subtract)
                    nc.vector.tensor_scalar_add(out=tg1, in0=tg1, scalar1=EPS)
                    # rsqrt(v) on DVE: quake seed + 1 Newton step
                    nc.vector.tensor_scalar(out=tg2.bitcast(I32), in0=tg1.bitcast(I32),
                                            scalar1=1, scalar2=None,
                                            op0=mybir.AluOpType.logical_shift_right)
                    nc.vector.tensor_scalar(out=tg2.bitcast(I32), in0=tg2.bitcast(I32),
                                            scalar1=-1, scalar2=0x5f3759df,
                                            op0=mybir.AluOpType.mult,
                                            op1=mybir.AluOpType.add)
                    nc.vector.tensor_mul(out=tg3, in0=tg2, in1=tg2)
                    nc.vector.tensor_mul(out=tg3, in0=tg3, in1=tg1)
                    nc.vector.tensor_scalar(out=tg4, in0=tg3, scalar1=-0.5,
                                            scalar2=1.5, op0=mybir.AluOpType.mult,
                                            op1=mybir.AluOpType.add)
                    nc.vector.tensor_mul(out=tg2, in0=tg2, in1=tg4)
                    nc.vector.tensor_copy(out=grs[:, 1:2], in_=tg2)
                    grs_t.append(grs)

                mc2 = pspre.tile([128, 2, 2], F32, tag="mcrs", name="mcrs")
                for t in range(2):
                    nc.tensor.matmul(mc2[:, t, :], lhsT=igt_sb[t],
                                     rhs=grs_t[t], start=True, stop=True)
                nc.vector.tensor_mul(out=a2, in0=vecs2_sb[:, :, VG], in1=mc2[:, :, 1])
                nc.vector.tensor_mul(out=b2, in0=mc2[:, :, 0], in1=a2)
                nc.vector.tensor_tensor(out=b2, in0=vecs2_sb[:, :, VB],
                                        in1=b2, op=mybir.AluOpType.subtract)

                # fold GN scale into Wv rows (split ACT/DVE); w2 = a (.) Wk^T bq
                nc.scalar.activation(out=wv_pk[:, 0, :], in_=wraw_v[0],
                                     func=mybir.ActivationFunctionType.Identity,
                                     scale=a_t[0])
                nc.vector.tensor_scalar_mul(out=wv_pk[:, 1, :], in0=wraw_v[1],
                                            scalar1=a_t[1])
                nc.vector.tensor_mul(out=w22, in0=wt_ps2, in1=a2)
                # Hq = a (.) x + b from the fp8 query chunk (chunk 0 after rotation)
                nc.gpsimd.tensor_scalar(out=hq_bf[0], in0=x_pk[:, 0, 0, :],
                                        scalar1=a_t[0], scalar2=b_t[0],
                                        op0=mybir.AluOpType.mult,
                                        op1=mybir.AluOpType.add)
                nc.vector.tensor_scalar(out=hq_bf[1], in0=x_pk[:, 0, 1, :],
                                        scalar1=a_t[1], scalar2=b_t[1],
                                        op0=mybir.AluOpType.mult,
                                        op1=mybir.AluOpType.add)

            # ---- cv, wpcv, G'' ----
            with tc.tile_pool(name="psgen", bufs=1, space="PSUM") as psgen:
                for m in range(2):
                    cp = psgen.tile([128, 1], F32, tag="cps", name="cps", bufs=1)
                    for t in range(2):
                        nc.tensor.matmul(cp, lhsT=wraw_v[t][:, m * 128:(m + 1) * 128],
                                         rhs=b_t[t], start=(t == 0), stop=(t == 1))
                    nc.vector.tensor_tensor(out=cv[m], in0=cp,
                                            in1=vecs_t[m][:, VBV:VBV + 1],
                                            op=mybir.AluOpType.add)
                    nc.vector.tensor_copy(out=cvbf[:, m:m + 1], in_=cv[m])
                # wpcv = Wp @ cv; bpv = bp + wpcv folds into the residual prep
                w_ps = psgen.tile([128, 2], F32, tag="wps", name="wps")
                for m in range(2):
                    for ct in range(2):
                        nc.tensor.matmul(w_ps[:, m:m + 1],
                                         lhsT=wpT_sb[:, ct, m * 128:(m + 1) * 128],
                                         rhs=cvbf[:, ct:ct + 1],
                                         start=(ct == 0), stop=(ct == 1))
                nc.vector.tensor_tensor(out=bpv2, in0=w_ps, in1=vecs2_sb[:, :, VBP],
                                        op=mybir.AluOpType.add)
                # residual prep in place: xqr = xq + bp + Wp@cv (gpsimd, off path)
                for t in range(2):
                    nc.gpsimd.tensor_scalar_add(out=xq_f[t], in0=xq_f[t],
                                                scalar1=bpv2[:, t:t + 1])

                for ct in range(2):
                    g_ps = psgen.tile([128, NQ], F32, tag="gps", name="gps", bufs=2)
                    for qh in range(2):
                        for cs in range(2):
                            nc.tensor.matmul(g_ps[:, qh * 512:(qh + 1) * 512],
                                             lhsT=m2_sb[cs][:, ct * 128:(ct + 1) * 128],
                                             rhs=hq_bf[cs][:, qh * 512:(qh + 1) * 512],
                                             start=(cs == 0), stop=(cs == 1))
                    dst = g_pk[:, ct, :]
                    if ct == 0:
                        nc.scalar.activation(out=dst, in_=g_ps,
                                             func=mybir.ActivationFunctionType.Identity,
                                             bias=w2[ct], scale=a_t[ct])
                    else:
                        nc.vector.tensor_scalar(out=dst, in0=g_ps, scalar1=a_t[ct],
                                                scalar2=w2[ct],
                                                op0=mybir.AluOpType.mult,
                                                op1=mybir.AluOpType.add)

            def x_lhsT(kt):  # [128, 2, 128] channel-packed key-tile slice
                return x_pk[:, kt // 8, :, (kt % 8) * 128:(kt % 8 + 1) * 128]

            o_sb = [attn.tile([128, C], BF16, tag=f"o{jq}", name=f"o{jq}")
                    for jq in range(NQT)]
            ot_bf = attn.tile([128, 2, NQ], BF16, tag="otbf", name="otbf")
            y_sb = [attn.tile([128, NQ], BF16, tag=f"y{t}", name=f"y{t}") for t in range(2)]

            # ---- S^T -> exp (ACT|DVE), V, and 2 early O chains ----
            with tc.tile_pool(name="ob01", bufs=1, space="PSUM") as ob01:
                ob_early = [ob01.tile([128, C + 1], F32, tag=f"obe{i}", name=f"obe{i}")
                            for i in range(2)]

                def chain_mm(ob, jq, j):
                    nc.tensor.matmul(ob,
                                     lhsT=pt[j][:, :, jq * 128:(jq + 1) * 128],
                                     rhs=vt1[j][:, :, 0:C + 1],
                                     start=(j == 0), stop=(j == NJ - 1),
                                     perf_mode=mybir.MatmulPerfMode.DoubleRow)

                with (
                    tc.tile_pool(name="pss", bufs=2, space="PSUM") as pss,
                    tc.tile_pool(name="psv", bufs=2, space="PSUM") as psv,
                ):
                    for j in range(NJ):
                        for s in range(2):
                            kt = 2 * j + s
                            sp = pss.tile([128, NQ], F32, tag="s", name="s")
                            for h in range(2):
                                nc.tensor.matmul(sp[:, h * 512:(h + 1) * 512],
                                                 lhsT=x_lhsT(kt),
                                                 rhs=g_pk[:, :, h * 512:(h + 1) * 512],
                                                 start=True, stop=True,
                                                 perf_mode=mybir.MatmulPerfMode.DoubleRow)
                            if s == 0:
                                nc.scalar.activation(out=pt[j][:, s, :], in_=sp, bias=ebias,
                                                     func=mybir.ActivationFunctionType.Exp,
                                                     scale=SCALE)
                            else:
                                nc.vector.tensor_scalar(
                                    out=pt[j][:, s, :].bitcast(U8), in0=sp,
                                    scalar1=float(EXP_A * SCALE),
                                    scalar2=float(EXP_B + LOGIT_BIAS * EXP_A),
                                    op0=mybir.AluOpType.mult, op1=mybir.AluOpType.add)
                        # V block j rides the S loop
                        vp = psv.tile([128, 2, C], F32, tag="vps", name="vp")
                        for s in range(2):
                            kt = 2 * j + s
                            nc.tensor.matmul(vp[:, s, :],
                                             lhsT=x_lhsT(kt),
                                             rhs=wv_pk,
                                             start=True, stop=True,
                                             perf_mode=mybir.MatmulPerfMode.DoubleRow)
                        if j in ACT_V_JS:
                            nc.scalar.copy(out=vt1[j][:, :, 0:C], in_=vp)
                        else:
                            nc.vector.tensor_copy(out=vt1[j][:, :, 0:C], in_=vp)
                        # O chains 0,1 trail one j behind (their exps are done)
                        if j >= 1:
                            chain_mm(ob_early[0], 0, j - 1)
                            chain_mm(ob_early[1], 1, j - 1)
                    chain_mm(ob_early[0], 0, NJ - 1)
                    chain_mm(ob_early[1], 1, NJ - 1)

                # ---- remaining O chains, DMA transposes, proj, residual, store ----
                with (
                    tc.tile_pool(name="ob2", bufs=2, space="PSUM") as ob2,
                    tc.tile_pool(name="psy", bufs=2, space="PSUM") as psy,
                ):
                    def evac_chain(ob, jq):
                        rec = small.tile([128, 1], F32, tag="rec2", name="rec2", bufs=4)
                        nc.vector.reciprocal(out=rec, in_=ob[:, C:C + 1])
                        nc.scalar.activation(out=o_sb[jq], in_=ob[:, 0:C],
                                             func=mybir.ActivationFunctionType.Identity,
                                             scale=rec)

                    def transpose_block(jq):
                        for ct in range(2):
                            q = nc.sync if (jq + ct) % 2 == 0 else nc.scalar
                            q.dma_start_transpose(
                                out=ot_bf[:, ct, jq * 128:(jq + 1) * 128],
                                in_=o_sb[jq][:, ct * 128:(ct + 1) * 128])

                    def proj_block(n):
                        for m in range(2):
                            yp = psy.tile([128, 512], F32, tag="yps", name="yps")
                            for ct in range(2):
                                nc.tensor.matmul(yp,
                                                 lhsT=wpT_sb[:, ct, m * 128:(m + 1) * 128],
                                                 rhs=ot_bf[:, ct, n * 512:(n + 1) * 512],
                                                 start=(ct == 0), stop=(ct == 1))
                            nc.vector.tensor_tensor(
                                out=y_sb[m][:, n * 512:(n + 1) * 512], in0=yp,
                                in1=xq_f[m][:, n * 512:(n + 1) * 512],
                                op=mybir.AluOpType.add)
                            q = nc.sync if (m + n) % 2 == 0 else nc.scalar
                            q.dma_start(out=y[m, n], in_=y_sb[m][:, n * 512:(n + 1) * 512])

                    evac_chain(ob_early[0], 0)
                    evac_chain(ob_early[1], 1)
                    transpose_block(0)
                    transpose_block(1)

                    obs = {}
                    for pair in ((2, 3), (4, 5), (6, 7)):
                        for jq in pair:
                            obs[jq] = ob2.tile([128, C + 1], F32, tag="ob2", name="ob2")
                        for j in range(NJ):
                            for jq in pair:
                                chain_mm(obs[jq], jq, j)
                        if pair == (4, 5):
                            proj_block(0)
                        for jq in pair:
                            evac_chain(obs[jq], jq)
                            transpose_block(jq)
                    proj_block(1)

    nc.compile()
    return nc


def bq_half(bq2, ot):
    return bq2[:, ot:ot + 1]


_NC_CACHE = None


def _get_nc():
    global _NC_CACHE
    if _NC_CACHE is None:
        _NC_CACHE = build_nc()
    return _NC_CACHE


def make_in_maps(inputs):
    x = np.ascontiguousarray(np.asarray(inputs["x"], np.float32))
    xf = x.reshape(B, C, N)
    xf_bf = xf.astype(ml_dtypes.float8_e4m3)
    group = np.arange(C) // (C // G)  # channel -> group
    ig = np.zeros((2, 128, GH), np.float32)
    igt = np.zeros((2, GH, 128), np.float32)
    for c in range(C):
        t = c // 128
        ig[t, c % 128, group[c] - t * GH] = 1.0 / (C // G)
        igt[t, group[c] - t * GH, c % 128] = 1.0
    vecs = np.zeros((2, 128, 5), np.float32)
    for t in range(2):
        sl = slice(t * 128, (t + 1) * 128)
        vecs[t, :, VG] = np.asarray(inputs["gn_gamma"])[sl]
        vecs[t, :, VB] = np.asarray(inputs["gn_beta"])[sl]
        vecs[t, :, VBQ] = np.asarray(inputs["bq"])[sl]
        vecs[t, :, VBV] = np.asarray(inputs["bv"])[sl]
        vecs[t, :, VBP] = np.asarray(inputs["bp"])[sl]
    wpT = np.ascontiguousarray(
        np.asarray(inputs["Wp"], np.float32).T.reshape(2, 128, C)
    ).astype(ml_dtypes.bfloat16)
    common = {
        "wqnt": np.ascontiguousarray(np.asarray(inputs["Wq"], ml_dtypes.bfloat16)),
        "wknt": np.ascontiguousarray(np.asarray(inputs["Wk"], ml_dtypes.bfloat16)),
        "wvT": np.ascontiguousarray(np.asarray(inputs["Wv"], np.float32).T),
        "wpT": wpT,
        "vecs": np.ascontiguousarray(vecs.transpose(1, 0, 2)),
        "ig": ig, "igt": igt,
    }
    in_maps = []
    for core in range(8):
        b, ch = core // 4, core % 4
        xb_cm = xf_bf[b].reshape(2, 128, 4, 1024).transpose(2, 1, 0, 3).reshape(4, 128, 2048)
        # rotate so the core's query chunk sits at index 0 (key order is
        # permutation-invariant through S -> P -> V -> O)
        rot = [(ch + i) % 4 for i in range(4)]
        in_maps.append({
            "xb": np.ascontiguousarray(xb_cm[rot]),
            "xq": np.ascontiguousarray(xf[b][:, ch * NQ:(ch + 1) * NQ]),
            **common,
        })
    return in_maps, x


def run(inputs, trace=False, tmpdir=None):
    nc = _get_nc()
    in_maps, x = make_in_maps(inputs)
    res = run_bass_kernel_spmd(nc, in_maps, core_ids=list(range(8)),
                               trace=trace, tmpdir=tmpdir)
    out = np.empty((B, C, N), np.float32)
    for core in range(8):
        b, ch = core // 4, core % 4
        yc = np.asarray(res.results[core]["y"], np.float32)  # [2, 2, 128, 512]
        out[b][:, ch * NQ:(ch + 1) * NQ] = yc.transpose(0, 2, 1, 3).reshape(C, NQ)
    return out.reshape(B, C, 16, 16, 16), res


def kernel(**inputs) -> np.ndarray:
    out, _ = run(inputs, trace=False)
    return out


# revision 9
# speedup vs baseline: 1.0938x; 1.0938x over previous
"""AttnBlock3d (GroupNorm -> QKV -> softmax attention -> proj -> residual) on 8 trn2 cores.

Sharding: 8 shards = batch (2) x query-chunk (4 x 1024 tokens). Each core receives the
full batch slice (for GN stats and K/V) plus its query chunk; per-core difference is
entirely in the input data, so one SPMD NEFF runs on all 8 cores with no collectives.
Host gathers the per-core [C, 1024] outputs back into [2, C, 16, 16, 16].

v2 structure (vs the 83us baseline):
- Startup: xb chunks stream on the sync HWDGE queue (staggered arrival feeds the
  GN stats as they land), weights on the scalar HWDGE queue, gpsimd does no DMA.
  xq (f32 residual) is only needed at the end and streams after xb.
- Host rotates the xb chunk axis per core so chunk 0 is always the core's query
  chunk (key order is permutation-invariant through S->P->V->O); Hq is then built
  from the fp8 x_pk directly, taking xq off the critical path.
- K and Q are never materialized: S^T = x^T G'' with G'' folded from the GN affine,
  bq, and Wq^T Wk (as in the baseline).
- S loop is evac-bound (ACT exact exp / DVE Schraudolph split + single-op V evacs,
  balanced by measured rates); two of the eight O accumulation chains ride the
  S loop's spare PE cycles in the 2 leftover PSUM banks.
- O^T via PE transposes evacuated straight to bf16; Wp stays bf16 (no fp8 2^13
  scale trick) and Wp@cv + bp fold into the final y evac, so the separate
  cv-add and residual-prep passes disappear.
"""

import ml_dtypes
import numpy as np

import concourse.bacc as bacc
import concourse.mybir as mybir
import concourse.tile as tile
from concourse.bass_utils import run_bass_kernel_spmd

B = 2
C = 256
G = 32
N = 4096          # D*H*W tokens per batch
NQ = 1024         # query chunk per core
EPS = 1e-5
SCALE = 1.0 / 16.0  # C ** -0.5
F32 = mybir.dt.float32
BF16 = mybir.dt.bfloat16
FP8 = mybir.dt.float8e4
U8 = mybir.dt.uint8
I32 = mybir.dt.int32
NT = N // 128     # 32 key tiles
NJ = NT // 2      # 16 key-pair blocks
NQT = NQ // 128   # 8 query tiles per core
GH = G // 2
WARMUP_MMS = 14

# Schraudolph fast-exp: exp(x) ~= bitcast_fp8e4(uint8(x * 8*log2(e) + 55.63))
EXP_A = 8.0 * 1.4426950408889634
EXP_B = 56.0 - 0.37
LOGIT_BIAS = -3.0  # softmax shift: exp(s/16 - 3) keeps fp8/u8 in range

# j indices whose V evac goes to ACT (9 of 16; DVE takes the rest)
ACT_V_JS = {0, 2, 4, 5, 8, 10, 12, 14, 6}

# vecs layout along the free dim: gamma, beta, bq, bv, bp
VG, VB, VBQ, VBV, VBP = range(5)
# f32 pack layout: [wvT 2x256 | vecs 5x2 (v-major) | ig 2x16]
VEC_OFF = 512
IG_OFF = VEC_OFF + 10
FPK_W = IG_OFF + 2 * GH


def build_nc():
    nc = bacc.Bacc("TRN2", target_bir_lowering=False, debug=False, num_devices=8)

    # x channel-packed fp8: [chunk 4, 128, (s=2, n=1024)]; chunk 0 = query chunk
    xb = nc.dram_tensor("xb", [4, 128, 2048], FP8, kind="ExternalInput").ap()
    xq = nc.dram_tensor("xq", [C, NQ], F32, kind="ExternalInput").ap()
    # all weights packed into 2 tensors: each dma_start dispatch costs ~650ns
    # of issuing-engine queue time, so minimize the count
    bfpk = nc.dram_tensor("bfpk", [128, 3, 2, C], BF16, kind="ExternalInput").ap()
    fpk = nc.dram_tensor("fpk", [128, FPK_W], F32, kind="ExternalInput").ap()
    igt = nc.dram_tensor("igt", [2, GH, 128], F32, kind="ExternalInput").ap()
    y = nc.dram_tensor("y", [2, 2, 128, 512], BF16, kind="ExternalOutput").ap()

    from concourse.masks import make_identity

    with tile.TileContext(nc) as tc:
        with (
            tc.tile_pool(name="consts", bufs=1) as consts,
            tc.tile_pool(name="small", bufs=1) as small,
            tc.tile_pool(name="kqv", bufs=1) as kqv,
            tc.tile_pool(name="attn", bufs=1) as attn,
        ):
            # ---- input DMAs: everything on the sync queue (sync engine has no
            # compute; its queue absorbs the per-dispatch cost). xb chunks
            # first, staggered so stats eat them in arrival order ----
            x_pk = kqv.tile([128, 4, 2, 1024], FP8, tag="xpk", name="xpk")
            for c in range(4):
                nc.sync.dma_start(out=x_pk[:, c], in_=xb[c])
            bfpk_sb = consts.tile([128, 3, 2, C], BF16, tag="bfpk", name="bfpk")
            nc.sync.dma_start(out=bfpk_sb, in_=bfpk)
            fpk_sb = consts.tile([128, FPK_W], F32, tag="fpk", name="fpk")
            nc.sync.dma_start(out=fpk_sb, in_=fpk)
            igt_sb = [consts.tile([GH, 128], F32, tag=f"igt{t}", name=f"igt{t}")
                      for t in range(2)]
            for t in range(2):
                nc.sync.dma_start(out=igt_sb[t], in_=igt[t])
            xq_f = [kqv.tile([128, NQ], F32, tag=f"xqf{t}", name=f"xqf{t}") for t in range(2)]
            for t in range(2):
                nc.sync.dma_start(out=xq_f[t], in_=xq[t * 128:(t + 1) * 128, :])

            wq_nt = [bfpk_sb[:, 0, t, :] for t in range(2)]
            wk_nt = [bfpk_sb[:, 1, t, :] for t in range(2)]
            wpT_t = [bfpk_sb[:, 2, ct, :] for ct in range(2)]
            wraw_v = [fpk_sb[:, t * C:(t + 1) * C] for t in range(2)]
            ig_t = [fpk_sb[:, IG_OFF + GH * t: IG_OFF + GH * (t + 1)] for t in range(2)]

            def vcol2(v):  # [128, 2] both c-halves of small vec v
                return fpk_sb[:, VEC_OFF + 2 * v: VEC_OFF + 2 * v + 2]

            def vcol(v, t):  # [128, 1] half t of small vec v
                return fpk_sb[:, VEC_OFF + 2 * v + t: VEC_OFF + 2 * v + t + 1]

            # small SBUF constants on gpsimd (no DMA there, just compute)
            ident = consts.tile([128, 128], BF16, tag="ident", name="ident")
            warm_rhs = consts.tile([128, 512], BF16, tag="warm", name="warm")
            make_identity(nc, ident)
            nc.gpsimd.memset(warm_rhs, 0.25)
            ebias = small.tile([128, 1], F32, tag="ebias", name="ebias")
            nc.gpsimd.memset(ebias, LOGIT_BIAS)

            g_pk = kqv.tile([128, 2, NQ], FP8, tag="gpk", name="gpk")
            hq_bf = [kqv.tile([128, NQ], BF16, tag=f"hq{t}", name=f"hq{t}") for t in range(2)]
            m2_sb = [kqv.tile([128, C], BF16, tag=f"m2{t}", name=f"m2{t}") for t in range(2)]
            wv_pk = consts.tile([128, 2, C], FP8, tag="wvpk", name="wvpk")
            vt1 = [kqv.tile([128, 2, C + 16], FP8, tag=f"vt{j}", name=f"vt{j}")
                   for j in range(NJ)]
            pt = [attn.tile([128, 2, NQ], FP8, tag=f"pt{j}", name=f"pt{j}")
                  for j in range(NJ)]
            for j in range(NJ):
                nc.gpsimd.memset(vt1[j][:, :, C:C + 1], 1.0)

            a2 = small.tile([128, 2], F32, tag="a2", name="a2")
            b2 = small.tile([128, 2], F32, tag="b2", name="b2")
            w22 = small.tile([128, 2], F32, tag="w22", name="w22")
            a_t = [a2[:, t:t + 1] for t in range(2)]
            b_t = [b2[:, t:t + 1] for t in range(2)]
            w2 = [w22[:, t:t + 1] for t in range(2)]
            cv = [small.tile([128, 1], F32, tag=f"cv{m}", name=f"cv{m}") for m in range(2)]
            cvbf = small.tile([128, 2], BF16, tag="cvbf", name="cvbf")
            bpv2 = small.tile([128, 2], F32, tag="bpv2", name="bpv2")
            bq2 = small.tile([128, 2], BF16, tag="bq2", name="bq2")
            pdum = small.tile([32, 1], F32, tag="pdum", name="pdum")

            def xsg(t, sg):  # [128, 512] subgroup sg of c-tile t, chunk-major
                return x_pk[:, sg // 2, t, (sg % 2) * 512:(sg % 2 + 1) * 512]

            with tc.tile_pool(name="pspre", bufs=1, space="PSUM") as pspre:
                # PE warmup while DMAs stream; preload the exp ACT table set.
                wp_ps = pspre.tile([128, 512], F32, tag="warmps", name="warmps")
                for _ in range(WARMUP_MMS):
                    nc.tensor.matmul(wp_ps, lhsT=ident, rhs=warm_rhs, start=True, stop=True)
                nc.scalar.activation(out=pdum, in_=ident[0:32, 0:1],
                                     func=mybir.ActivationFunctionType.Exp, scale=1.0)

                # M2 = (Wq^T Wk) tiles: m2_sb[cs][p, f] = Mk[f, cs*128+p]
                for cs in range(2):
                    m2_ps = pspre.tile([128, C], F32, tag="m2ps", name="m2ps", bufs=2)
                    for ot in range(2):
                        nc.tensor.matmul(m2_ps, lhsT=wq_nt[ot][:, cs * 128:(cs + 1) * 128],
                                         rhs=wk_nt[ot], start=(ot == 0), stop=(ot == 1))
                    if cs == 0:
                        nc.scalar.copy(out=m2_sb[cs], in_=m2_ps)
                    else:
                        nc.vector.tensor_copy(out=m2_sb[cs], in_=m2_ps)
                # w~ = Wk^T bq (per c-tile), later scaled by a into w2
                nc.vector.tensor_copy(out=bq2, in_=vcol2(VBQ))
                wt_ps2 = pspre.tile([128, 2], F32, tag="wtps", name="wtps")
                wt_ps = [wt_ps2[:, ct:ct + 1] for ct in range(2)]
                for ct in range(2):
                    for ot in range(2):
                        nc.tensor.matmul(wt_ps[ct],
                                         lhsT=wk_nt[ot][:, ct * 128:(ct + 1) * 128],
                                         rhs=bq_half(bq2, ot), start=(ot == 0), stop=(ot == 1))

                # keep the PE busy across the stats window (HAM stays warm)
                for _ in range(4):
                    nc.tensor.matmul(wp_ps, lhsT=ident, rhs=warm_rhs, start=True, stop=True)

                # ---- GN stats in chunk-arrival order ----
                # c-tile 0 + chunk 3 of c-tile 1: DVE bn_stats (10 subgroups);
                # chunks 0-2 of c-tile 1: ACT Identity/Square accumulations.
                st = [small.tile([128, 2], F32, tag=f"st{t}", name=f"st{t}") for t in range(2)]

                stats8a = small.tile([128, 8, 6], F32, tag="stats8a", name="stats8a")
                mva = small.tile([128, 2], F32, tag="mva", name="mva")
                stats2 = small.tile([128, 2, 6], F32, tag="stats2", name="stats2")
                mv = small.tile([128, 2], F32, tag="mv", name="mv")
                acc = small.tile([128, 3, 2], F32, tag="acc", name="acc")

                for i, sg in enumerate(range(8)):
                    nc.vector.bn_stats(out=stats8a[:, i, :], in_=xsg(0, sg))
                for i in range(2):
                    nc.vector.bn_stats(out=stats2[:, i, :], in_=xsg(1, 6 + i))
                for i in range(3):
                    rng = x_pk[:, i, 1, :]
                    junk = small.tile([128, 1024], BF16, tag="junk", name="junk", bufs=2)
                    nc.scalar.activation(out=junk, in_=rng,
                                         func=mybir.ActivationFunctionType.Identity,
                                         accum_out=acc[:, i, 0:1])
                    nc.scalar.activation(out=junk, in_=rng,
                                         func=mybir.ActivationFunctionType.Square,
                                         accum_out=acc[:, i, 1:2])
                nc.vector.bn_aggr(out=mva, in_=stats8a)
                nc.vector.tensor_copy(out=st[0][:, 0:1], in_=mva[:, 0:1])
                nc.vector.tensor_mul(out=st[0][:, 1:2], in0=mva[:, 0:1], in1=mva[:, 0:1])
                nc.vector.tensor_add(out=st[0][:, 1:2], in0=st[0][:, 1:2], in1=mva[:, 1:2])

                nc.vector.bn_aggr(out=mv, in_=stats2)
                n_dve = float(2 * 1024)
                n_tot = float(N)
                sums = small.tile([128, 2], F32, tag="sums", name="sums")
                nc.vector.tensor_tensor(out=sums, in0=acc[:, 0, :],
                                        in1=acc[:, 1, :], op=mybir.AluOpType.add)
                nc.vector.tensor_tensor(out=sums, in0=sums,
                                        in1=acc[:, 2, :], op=mybir.AluOpType.add)
                # E[x] = (mean_dve*n_dve + sum_act)/N
                nc.vector.scalar_tensor_tensor(out=st[1][:, 0:1], in0=mv[:, 0:1],
                                               scalar=n_dve, in1=sums[:, 0:1],
                                               op0=mybir.AluOpType.mult,
                                               op1=mybir.AluOpType.add)
                nc.vector.tensor_scalar_mul(out=st[1][:, 0:1], in0=st[1][:, 0:1],
                                            scalar1=1.0 / n_tot)
                # E[x^2] = ((var+mean^2)*n_dve + sumsq_act)/N
                nc.vector.tensor_mul(out=st[1][:, 1:2], in0=mv[:, 0:1], in1=mv[:, 0:1])
                nc.vector.tensor_add(out=st[1][:, 1:2], in0=st[1][:, 1:2], in1=mv[:, 1:2])
                nc.vector.scalar_tensor_tensor(out=st[1][:, 1:2], in0=st[1][:, 1:2],
                                               scalar=n_dve, in1=sums[:, 1:2],
                                               op0=mybir.AluOpType.mult,
                                               op1=mybir.AluOpType.add)
                nc.vector.tensor_scalar_mul(out=st[1][:, 1:2], in0=st[1][:, 1:2],
                                            scalar1=1.0 / n_tot)

                grs_t = []
                for t in range(2):
                    # per-tile group stats -> rsqrt chain
                    ps_g = pspre.tile([GH, 2], F32, tag="gst", name="gst", bufs=2)
                    nc.tensor.matmul(ps_g, lhsT=ig_t[t],
                                     rhs=st[t], start=True, stop=True)
                    tg1 = small.tile([GH, 1], F32, tag=f"tg1{t}", name=f"tg1{t}")
                    tg2 = small.tile([GH, 1], F32, tag=f"tg2{t}", name=f"tg2{t}")
                    tg3 = small.tile([GH, 1], F32, tag=f"tg3{t}", name=f"tg3{t}")
                    tg4 = small.tile([GH, 1], F32, tag=f"tg4{t}", name=f"tg4{t}")
                    grs = small.tile([GH, 2], F32, tag=f"grs{t}", name=f"grs{t}")
                    nc.vector.tensor_copy(out=grs[:, 0:1], in_=ps_g[:, 0:1])
                    nc.vector.tensor_mul(out=tg1, in0=grs[:, 0:1], in1=grs[:, 0:1])
                    nc.vector.tensor_tensor(out=tg1, in0=ps_g[:, 1:2], in1=tg1,
                                            op=mybir.AluOpType.subtract)
                    nc.vector.tensor_scalar_add(out=tg1, in0=tg1, scalar1=EPS)
                    # rsqrt(v) on DVE: quake seed + 1 Newton step
                    nc.vector.tensor_scalar(out=tg2.bitcast(I32), in0=tg1.bitcast(I32),
                                            scalar1=1, scalar2=None,
                                            op0=mybir.AluOpType.logical_shift_right)
                    nc.vector.tensor_scalar(out=tg2.bitcast(I32), in0=tg2.bitcast(I32),
                                            scalar1=-1, scalar2=0x5f3759df,
                                            op0=mybir.AluOpType.mult,
                                            op1=mybir.AluOpType.add)
                    nc.vector.tensor_mul(out=tg3, in0=tg2, in1=tg2)
                    nc.vector.tensor_mul(out=tg3, in0=tg3, in1=tg1)
                    nc.vector.tensor_scalar(out=tg4, in0=tg3, scalar1=-0.5,
                                            scalar2=1.5, op0=mybir.AluOpType.mult,
                                            op1=mybir.AluOpType.add)
                    nc.vector.tensor_mul(out=tg2, in0=tg2, in1=tg4)
                    nc.vector.tensor_copy(out=grs[:, 1:2], in_=tg2)
                    grs_t.append(grs)

                mc2 = pspre.tile([128, 2, 2], F32, tag="mcrs", name="mcrs")
                for t in range(2):
                    nc.tensor.matmul(mc2[:, t, :], lhsT=igt_sb[t],
                                     rhs=grs_t[t], start=True, stop=True)
                nc.vector.tensor_mul(out=a2, in0=vcol2(VG), in1=mc2[:, :, 1])
                nc.vector.tensor_mul(out=b2, in0=mc2[:, :, 0], in1=a2)
                nc.vector.tensor_tensor(out=b2, in0=vcol2(VB),
                                        in1=b2, op=mybir.AluOpType.subtract)

                # fold GN scale into Wv rows (split ACT/DVE); w2 = a (.) Wk^T bq
                nc.scalar.activation(out=wv_pk[:, 0, :], in_=wraw_v[0],
                                     func=mybir.ActivationFunctionType.Identity,
                                     scale=a_t[0])
                nc.vector.tensor_scalar_mul(out=wv_pk[:, 1, :], in0=wraw_v[1],
                                            scalar1=a_t[1])
                nc.vector.tensor_mul(out=w22, in0=wt_ps2, in1=a2)
                # Hq = a (.) x + b from the fp8 query chunk (chunk 0 after rotation)
                nc.gpsimd.tensor_scalar(out=hq_bf[0], in0=x_pk[:, 0, 0, :],
                                        scalar1=a_t[0], scalar2=b_t[0],
                                        op0=mybir.AluOpType.mult,
                                        op1=mybir.AluOpType.add)
                nc.vector.tensor_scalar(out=hq_bf[1], in0=x_pk[:, 0, 1, :],
                                        scalar1=a_t[1], scalar2=b_t[1],
                                        op0=mybir.AluOpType.mult,
                                        op1=mybir.AluOpType.add)

            # ---- cv, wpcv, G'' ----
            with tc.tile_pool(name="psgen", bufs=1, space="PSUM") as psgen:
                for m in range(2):
                    cp = psgen.tile([128, 1], F32, tag="cps", name="cps", bufs=1)
                    for t in range(2):
                        nc.tensor.matmul(cp, lhsT=wraw_v[t][:, m * 128:(m + 1) * 128],
                                         rhs=b_t[t], start=(t == 0), stop=(t == 1))
                    nc.vector.tensor_tensor(out=cv[m], in0=cp,
                                            in1=vcol(VBV, m),
                                            op=mybir.AluOpType.add)
                    nc.vector.tensor_copy(out=cvbf[:, m:m + 1], in_=cv[m])
                # wpcv = Wp @ cv; bpv = bp + wpcv folds into the residual prep
                w_ps = psgen.tile([128, 2], F32, tag="wps", name="wps")
                for m in range(2):
                    for ct in range(2):
                        nc.tensor.matmul(w_ps[:, m:m + 1],
                                         lhsT=wpT_t[ct][:, m * 128:(m + 1) * 128],
                                         rhs=cvbf[:, ct:ct + 1],
                                         start=(ct == 0), stop=(ct == 1))
                nc.vector.tensor_tensor(out=bpv2, in0=w_ps, in1=vcol2(VBP),
                                        op=mybir.AluOpType.add)
                for ct in range(2):
                    g_ps = psgen.tile([128, NQ], F32, tag="gps", name="gps", bufs=2)
                    for qh in range(2):
                        for cs in range(2):
                            nc.tensor.matmul(g_ps[:, qh * 512:(qh + 1) * 512],
                                             lhsT=m2_sb[cs][:, ct * 128:(ct + 1) * 128],
                                             rhs=hq_bf[cs][:, qh * 512:(qh + 1) * 512],
                                             start=(cs == 0), stop=(cs == 1))
                    dst = g_pk[:, ct, :]
                    if ct == 0:
                        nc.scalar.activation(out=dst, in_=g_ps,
                                             func=mybir.ActivationFunctionType.Identity,
                                             bias=w2[ct], scale=a_t[ct])
                    else:
                        nc.vector.tensor_scalar(out=dst, in0=g_ps, scalar1=a_t[ct],
                                                scalar2=w2[ct],
                                                op0=mybir.AluOpType.mult,
                                                op1=mybir.AluOpType.add)

            def x_lhsT(kt):  # [128, 2, 128] channel-packed key-tile slice
                return x_pk[:, kt // 8, :, (kt % 8) * 128:(kt % 8 + 1) * 128]

            o_sb = [attn.tile([128, C], BF16, tag=f"o{jq}", name=f"o{jq}")
                    for jq in range(NQT)]
            ot_bf = attn.tile([128, 2, NQ], BF16, tag="otbf", name="otbf")
            y_sb = [attn.tile([128, NQ], BF16, tag=f"y{t}", name=f"y{t}") for t in range(2)]

            # ---- S^T -> exp (ACT|DVE), V, and 2 early O chains ----
            with tc.tile_pool(name="ob01", bufs=1, space="PSUM") as ob01:
                ob_early = [ob01.tile([128, C + 1], F32, tag=f"obe{i}", name=f"obe{i}")
                            for i in range(2)]

                def chain_mm(ob, jq, j):
                    nc.tensor.matmul(ob,
                                     lhsT=pt[j][:, :, jq * 128:(jq + 1) * 128],
                                     rhs=vt1[j][:, :, 0:C + 1],
                                     start=(j == 0), stop=(j == NJ - 1),
                                     perf_mode=mybir.MatmulPerfMode.DoubleRow)

                with (
                    tc.tile_pool(name="pss", bufs=2, space="PSUM") as pss,
                    tc.tile_pool(name="psv", bufs=2, space="PSUM") as psv,
                ):
                    for j in range(NJ):
                        for s in range(2):
                            kt = 2 * j + s
                            sp = pss.tile([128, NQ], F32, tag="s", name="s")
                            for h in range(2):
                                nc.tensor.matmul(sp[:, h * 512:(h + 1) * 512],
                                                 lhsT=x_lhsT(kt),
                                                 rhs=g_pk[:, :, h * 512:(h + 1) * 512],
                                                 start=True, stop=True,
                                                 perf_mode=mybir.MatmulPerfMode.DoubleRow)
                            if s == 0:
                                nc.scalar.activation(out=pt[j][:, s, :], in_=sp, bias=ebias,
                                                     func=mybir.ActivationFunctionType.Exp,
                                                     scale=SCALE)
                            else:
                                nc.vector.tensor_scalar(
                                    out=pt[j][:, s, :].bitcast(U8), in0=sp,
                                    scalar1=float(EXP_A * SCALE),
                                    scalar2=float(EXP_B + LOGIT_BIAS * EXP_A),
                                    op0=mybir.AluOpType.mult, op1=mybir.AluOpType.add)
                        # V block j rides the S loop
                        vp = psv.tile([128, 2, C], F32, tag="vps", name="vp")
                        for s in range(2):
                            kt = 2 * j + s
                            nc.tensor.matmul(vp[:, s, :],
                                             lhsT=x_lhsT(kt),
                                             rhs=wv_pk,
                                             start=True, stop=True,
                                             perf_mode=mybir.MatmulPerfMode.DoubleRow)
                        if j in ACT_V_JS:
                            nc.scalar.copy(out=vt1[j][:, :, 0:C], in_=vp)
                        else:
                            nc.vector.tensor_copy(out=vt1[j][:, :, 0:C], in_=vp)
                        # O chains 0,1 trail one j behind (their exps are done)
                        if j >= 1:
                            chain_mm(ob_early[0], 0, j - 1)
                            chain_mm(ob_early[1], 1, j - 1)
                    chain_mm(ob_early[0], 0, NJ - 1)
                    chain_mm(ob_early[1], 1, NJ - 1)

                # ---- remaining O chains, DMA transposes, proj, residual, store ----
                with (
                    tc.tile_pool(name="ob2", bufs=2, space="PSUM") as ob2,
                    tc.tile_pool(name="pst", bufs=2, space="PSUM") as pst,
                    tc.tile_pool(name="psy", bufs=2, space="PSUM") as psy,
                ):
                    def evac_chain(ob, jq):
                        rec = small.tile([128, 1], F32, tag="rec2", name="rec2", bufs=4)
                        nc.vector.reciprocal(out=rec, in_=ob[:, C:C + 1])
                        nc.scalar.activation(out=o_sb[jq], in_=ob[:, 0:C],
                                             func=mybir.ActivationFunctionType.Identity,
                                             scale=rec)

                    def transpose_block(jq):
                        for ct in range(2):
                            tp = pst.tile([128, 128], BF16, tag="tp", name="tp")
                            nc.tensor.transpose(tp, o_sb[jq][:, ct * 128:(ct + 1) * 128],
                                                ident)
                            dst = ot_bf[:, ct, jq * 128:(jq + 1) * 128]
                            if (jq + ct) % 2 == 0:
                                nc.scalar.copy(out=dst, in_=tp)
                            else:
                                nc.vector.tensor_copy(out=dst, in_=tp)

                    def proj_block(n):
                        for m in range(2):
                            yp = psy.tile([128, 512], F32, tag="yps", name="yps")
                            for ct in range(2):
                                nc.tensor.matmul(yp,
                                                 lhsT=wpT_t[ct][:, m * 128:(m + 1) * 128],
                                                 rhs=ot_bf[:, ct, n * 512:(n + 1) * 512],
                                                 start=(ct == 0), stop=(ct == 1))
                            # y = yp + (bp + Wp@cv) + xq  (residual bias folded here)
                            nc.vector.scalar_tensor_tensor(
                                out=y_sb[m][:, n * 512:(n + 1) * 512], in0=yp,
                                scalar=bpv2[:, m:m + 1],
                                in1=xq_f[m][:, n * 512:(n + 1) * 512],
                                op0=mybir.AluOpType.add, op1=mybir.AluOpType.add)
                            nc.sync.dma_start(out=y[m, n],
                                              in_=y_sb[m][:, n * 512:(n + 1) * 512])

                    evac_chain(ob_early[0], 0)
                    evac_chain(ob_early[1], 1)

                    obs = {}

                    def chains(pair):
                        for jq in pair:
                            obs[jq] = ob2.tile([128, C + 1], F32, tag="ob2", name="ob2")
                        for j in range(NJ):
                            for jq in pair:
                                chain_mm(obs[jq], jq, j)

                    chains((2, 3))
                    transpose_block(0)
                    transpose_block(1)
                    evac_chain(obs[2], 2)
                    evac_chain(obs[3], 3)
                    chains((4, 5))
                    transpose_block(2)
                    transpose_block(3)
                    evac_chain(obs[4], 4)
                    evac_chain(obs[5], 5)
                    proj_block(0)
                    chains((6, 7))
                    transpose_block(4)
                    transpose_block(5)
                    evac_chain(obs[6], 6)
                    evac_chain(obs[7], 7)
                    transpose_block(6)
                    transpose_block(7)
                    proj_block(1)

    nc.compile()
    return nc


def bq_half(bq2, ot):
    return bq2[:, ot:ot + 1]


_NC_CACHE = None


def _get_nc():
    global _NC_CACHE
    if _NC_CACHE is None:
        _NC_CACHE = build_nc()
    return _NC_CACHE


def make_in_maps(inputs):
    x = np.ascontiguousarray(np.asarray(inputs["x"], np.float32))
    xf = x.reshape(B, C, N)
    xf_bf = xf.astype(ml_dtypes.float8_e4m3)
    group = np.arange(C) // (C // G)  # channel -> group
    ig = np.zeros((2, 128, GH), np.float32)
    igt = np.zeros((2, GH, 128), np.float32)
    for c in range(C):
        t = c // 128
        ig[t, c % 128, group[c] - t * GH] = 1.0 / (C // G)
        igt[t, group[c] - t * GH, c % 128] = 1.0
    vecs = np.zeros((2, 128, 5), np.float32)
    for t in range(2):
        sl = slice(t * 128, (t + 1) * 128)
        vecs[t, :, VG] = np.asarray(inputs["gn_gamma"])[sl]
        vecs[t, :, VB] = np.asarray(inputs["gn_beta"])[sl]
        vecs[t, :, VBQ] = np.asarray(inputs["bq"])[sl]
        vecs[t, :, VBV] = np.asarray(inputs["bv"])[sl]
        vecs[t, :, VBP] = np.asarray(inputs["bp"])[sl]
    # bf16 pack [128, 3, 2, C]: wq | wk | wpT, each [half, row-in-half, :]
    bfpk = np.empty((128, 3, 2, C), np.float32)
    bfpk[:, 0] = np.asarray(inputs["Wq"], np.float32).reshape(2, 128, C).transpose(1, 0, 2)
    bfpk[:, 1] = np.asarray(inputs["Wk"], np.float32).reshape(2, 128, C).transpose(1, 0, 2)
    bfpk[:, 2] = np.asarray(inputs["Wp"], np.float32).T.reshape(2, 128, C).transpose(1, 0, 2)
    # f32 pack [128, FPK_W]: wvT (t-major) | vecs (v-major) | ig
    fpk = np.empty((128, FPK_W), np.float32)
    fpk[:, 0:2 * C] = np.asarray(inputs["Wv"], np.float32).T.reshape(2, 128, C) \
        .transpose(1, 0, 2).reshape(128, 2 * C)
    fpk[:, VEC_OFF:VEC_OFF + 10] = vecs.transpose(1, 2, 0).reshape(128, 10)
    fpk[:, IG_OFF:] = ig.transpose(1, 0, 2).reshape(128, 2 * GH)
    common = {
        "bfpk": np.ascontiguousarray(bfpk.astype(ml_dtypes.bfloat16)),
        "fpk": np.ascontiguousarray(fpk),
        "igt": igt,
    }
    in_maps = []
    for core in range(8):
        b, ch = core // 4, core % 4
        xb_cm = xf_bf[b].reshape(2, 128, 4, 1024).transpose(2, 1, 0, 3).reshape(4, 128, 2048)
        # rotate so the core's query chunk sits at index 0 (key order is
        # permutation-invariant through S -> P -> V -> O)
        rot = [(ch + i) % 4 for i in range(4)]
        in_maps.append({
            "xb": np.ascontiguousarray(xb_cm[rot]),
            "xq": np.ascontiguousarray(xf[b][:, ch * NQ:(ch + 1) * NQ]),
            **common,
        })
    return in_maps, x


def run(inputs, trace=False, tmpdir=None):
    nc = _get_nc()
    in_maps, x = make_in_maps(inputs)
    res = run_bass_kernel_spmd(nc, in_maps, core_ids=list(range(8)),
                               trace=trace, tmpdir=tmpdir)
    out = np.empty((B, C, N), np.float32)
    for core in range(8):
        b, ch = core // 4, core % 4
        yc = np.asarray(res.results[core]["y"], np.float32)  # [2, 2, 128, 512]
        out[b][:, ch * NQ:(ch + 1) * NQ] = yc.transpose(0, 2, 1, 3).reshape(C, NQ)
    return out.reshape(B, C, 16, 16, 16), res


def kernel(**inputs) -> np.ndarray:
    out, _ = run(inputs, trace=False)
    return out


# revision 11
# speedup vs baseline: 1.2090x; 1.1053x over previous
"""AttnBlock3d (GroupNorm -> QKV -> softmax attention -> proj -> residual) on 8 trn2 cores.

Sharding: 8 shards = batch (2) x query-chunk (4 x 1024 tokens). Each core receives the
full batch slice (for GN stats and K/V) plus its query chunk; per-core difference is
entirely in the input data, so one SPMD NEFF runs on all 8 cores with no collectives.
Host gathers the per-core [C, 1024] outputs back into [2, C, 16, 16, 16].

v2 structure (vs the 83us baseline):
- Startup: xb chunks stream on the sync HWDGE queue (staggered arrival feeds the
  GN stats as they land), weights on the scalar HWDGE queue, gpsimd does no DMA.
  xq (f32 residual) is only needed at the end and streams after xb.
- Host rotates the xb chunk axis per core so chunk 0 is always the core's query
  chunk (key order is permutation-invariant through S->P->V->O); Hq is then built
  from the fp8 x_pk directly, taking xq off the critical path.
- K and Q are never materialized: S^T = x^T G'' with G'' folded from the GN affine,
  bq, and Wq^T Wk (as in the baseline).
- S loop is evac-bound (ACT exact exp / DVE Schraudolph split + single-op V evacs,
  balanced by measured rates); two of the eight O accumulation chains ride the
  S loop's spare PE cycles in the 2 leftover PSUM banks.
- O^T via PE transposes evacuated straight to bf16; Wp stays bf16 (no fp8 2^13
  scale trick) and Wp@cv + bp fold into the final y evac, so the separate
  cv-add and residual-prep passes disappear.
"""

import ml_dtypes
import numpy as np

import concourse.bacc as bacc
import concourse.mybir as mybir
import concourse.tile as tile
from concourse.bass_utils import run_bass_kernel_spmd

B = 2
C = 256
G = 32
N = 4096          # D*H*W tokens per batch
NQ = 1024         # query chunk per core
EPS = 1e-5
SCALE = 1.0 / 16.0  # C ** -0.5
F32 = mybir.dt.float32
BF16 = mybir.dt.bfloat16
FP8 = mybir.dt.float8e4
U8 = mybir.dt.uint8
I32 = mybir.dt.int32
NT = N // 128     # 32 key tiles
NJ = NT // 2      # 16 key-pair blocks
NQT = NQ // 128   # 8 query tiles per core
GH = G // 2
WARMUP_MMS = 14

# Schraudolph fast-exp: exp(x) ~= bitcast_fp8e4(uint8(x * 8*log2(e) + 55.63))
EXP_A = 8.0 * 1.4426950408889634
EXP_B = 56.0 - 0.37
LOGIT_BIAS = -3.0  # softmax shift: exp(s/16 - 3) keeps fp8/u8 in range

# j indices whose V evac goes to ACT (9 of 16; DVE takes the rest)
ACT_V_JS = {0, 2, 4, 5, 8, 10, 12, 14, 6}

# vecs layout along the free dim: gamma, beta, bq, bv, bp
VG, VB, VBQ, VBV, VBP = range(5)
# f32 pack layout: [wvT 2x256 | vecs 5x2 (v-major) | ig 2x16]
VEC_OFF = 512
IG_OFF = VEC_OFF + 10
FPK_W = IG_OFF + 2 * GH


def build_nc():
    nc = bacc.Bacc("TRN2", target_bir_lowering=False, debug=False, num_devices=8)

    # x channel-packed fp8: [chunk 4, 128, (s=2, n=1024)]; chunk 0 = query chunk
    xb = nc.dram_tensor("xb", [4, 128, 2048], FP8, kind="ExternalInput").ap()
    xq = nc.dram_tensor("xq", [C, NQ], F32, kind="ExternalInput").ap()
    # all weights packed into 2 tensors: each dma_start dispatch costs ~650ns
    # of issuing-engine queue time, so minimize the count
    bfpk = nc.dram_tensor("bfpk", [128, 3, 2, C], BF16, kind="ExternalInput").ap()
    fpk = nc.dram_tensor("fpk", [128, FPK_W], F32, kind="ExternalInput").ap()
    igt = nc.dram_tensor("igt", [2, GH, 128], F32, kind="ExternalInput").ap()
    y = nc.dram_tensor("y", [2, 2, 128, 512], BF16, kind="ExternalOutput").ap()

    from concourse.masks import make_identity

    with tile.TileContext(nc) as tc:
        with (
            tc.tile_pool(name="consts", bufs=1) as consts,
            tc.tile_pool(name="small", bufs=1) as small,
            tc.tile_pool(name="kqv", bufs=1) as kqv,
            tc.tile_pool(name="attn", bufs=1) as attn,
        ):
            # ---- input DMAs: everything on the sync queue (sync engine has no
            # compute; its queue absorbs the per-dispatch cost). xb chunks
            # first, staggered so stats eat them in arrival order ----
            x_pk = kqv.tile([128, 4, 2, 1024], FP8, tag="xpk", name="xpk")
            for c in range(4):
                nc.sync.dma_start(out=x_pk[:, c], in_=xb[c])
            bfpk_sb = consts.tile([128, 3, 2, C], BF16, tag="bfpk", name="bfpk")
            nc.sync.dma_start(out=bfpk_sb, in_=bfpk)
            fpk_sb = consts.tile([128, FPK_W], F32, tag="fpk", name="fpk")
            nc.sync.dma_start(out=fpk_sb, in_=fpk)
            igt_sb = [consts.tile([GH, 128], F32, tag=f"igt{t}", name=f"igt{t}")
                      for t in range(2)]
            for t in range(2):
                nc.sync.dma_start(out=igt_sb[t], in_=igt[t])
            xq_f = [kqv.tile([128, NQ], F32, tag=f"xqf{t}", name=f"xqf{t}") for t in range(2)]
            for t in range(2):
                nc.sync.dma_start(out=xq_f[t], in_=xq[t * 128:(t + 1) * 128, :])

            wq_nt = [bfpk_sb[:, 0, t, :] for t in range(2)]
            wk_nt = [bfpk_sb[:, 1, t, :] for t in range(2)]
            wpT_t = [bfpk_sb[:, 2, ct, :] for ct in range(2)]
            wraw_v = [fpk_sb[:, t * C:(t + 1) * C] for t in range(2)]
            ig_t = [fpk_sb[:, IG_OFF + GH * t: IG_OFF + GH * (t + 1)] for t in range(2)]

            def vcol2(v):  # [128, 2] both c-halves of small vec v
                return fpk_sb[:, VEC_OFF + 2 * v: VEC_OFF + 2 * v + 2]

            def vcol(v, t):  # [128, 1] half t of small vec v
                return fpk_sb[:, VEC_OFF + 2 * v + t: VEC_OFF + 2 * v + t + 1]

            # small SBUF constants on gpsimd (no DMA there, just compute)
            ident = consts.tile([128, 128], BF16, tag="ident", name="ident")
            warm_rhs = consts.tile([128, 512], BF16, tag="warm", name="warm")
            make_identity(nc, ident)
            nc.gpsimd.memset(warm_rhs, 0.25)
            ebias = small.tile([128, 1], F32, tag="ebias", name="ebias")
            nc.gpsimd.memset(ebias, LOGIT_BIAS)

            g_pk = kqv.tile([128, 2, NQ], FP8, tag="gpk", name="gpk")
            hq_bf = [kqv.tile([128, NQ], BF16, tag=f"hq{t}", name=f"hq{t}") for t in range(2)]
            m2_sb = [kqv.tile([128, C], BF16, tag=f"m2{t}", name=f"m2{t}") for t in range(2)]
            wv_pk = consts.tile([128, 2, C], FP8, tag="wvpk", name="wvpk")
            vt1 = [kqv.tile([128, 2, C + 16], FP8, tag=f"vt{j}", name=f"vt{j}")
                   for j in range(NJ)]
            pt = [attn.tile([128, 2, NQ], FP8, tag=f"pt{j}", name=f"pt{j}")
                  for j in range(NJ)]
            for j in range(NJ):
                nc.gpsimd.memset(vt1[j][:, :, C:C + 1], 1.0)

            a2 = small.tile([128, 2], F32, tag="a2", name="a2")
            b2 = small.tile([128, 2], F32, tag="b2", name="b2")
            w22 = small.tile([128, 2], F32, tag="w22", name="w22")
            a_t = [a2[:, t:t + 1] for t in range(2)]
            b_t = [b2[:, t:t + 1] for t in range(2)]
            w2 = [w22[:, t:t + 1] for t in range(2)]
            cv = [small.tile([128, 1], F32, tag=f"cv{m}", name=f"cv{m}") for m in range(2)]
            cvbf = small.tile([128, 2], BF16, tag="cvbf", name="cvbf")
            bpv2 = small.tile([128, 2], F32, tag="bpv2", name="bpv2")
            bq2 = small.tile([128, 2], BF16, tag="bq2", name="bq2")
            pdum = small.tile([32, 1], F32, tag="pdum", name="pdum")

            def xsg(t, sg):  # [128, 512] subgroup sg of c-tile t, chunk-major
                return x_pk[:, sg // 2, t, (sg % 2) * 512:(sg % 2 + 1) * 512]

            with tc.tile_pool(name="pspre", bufs=1, space="PSUM") as pspre:
                # PE warmup while DMAs stream; preload the exp ACT table set.
                wp_ps = pspre.tile([128, 512], F32, tag="warmps", name="warmps")
                for _ in range(WARMUP_MMS):
                    nc.tensor.matmul(wp_ps, lhsT=ident, rhs=warm_rhs, start=True, stop=True)
                nc.scalar.activation(out=pdum, in_=ident[0:32, 0:1],
                                     func=mybir.ActivationFunctionType.Exp, scale=1.0)
                nc.gpsimd.tensor_copy(out=bq2, in_=vcol2(VBQ))

                # ---- GN stats in chunk-arrival order ----
                # c-tile 0 + chunk 3 of c-tile 1: DVE bn_stats (10 subgroups);
                # chunks 0-2 of c-tile 1: ACT Identity/Square accumulations.
                st = [small.tile([128, 2], F32, tag=f"st{t}", name=f"st{t}") for t in range(2)]

                stats8a = small.tile([128, 8, 6], F32, tag="stats8a", name="stats8a")
                mva = small.tile([128, 2], F32, tag="mva", name="mva")
                stats2 = small.tile([128, 2, 6], F32, tag="stats2", name="stats2")
                mv = small.tile([128, 2], F32, tag="mv", name="mv")
                acc = small.tile([128, 3, 2], F32, tag="acc", name="acc")

                for i, sg in enumerate(range(8)):
                    nc.vector.bn_stats(out=stats8a[:, i, :], in_=xsg(0, sg))
                for i in range(2):
                    nc.vector.bn_stats(out=stats2[:, i, :], in_=xsg(1, 6 + i))
                for i in range(3):
                    rng = x_pk[:, i, 1, :]
                    junk = small.tile([128, 1024], BF16, tag="junk", name="junk", bufs=2)
                    nc.scalar.activation(out=junk, in_=rng,
                                         func=mybir.ActivationFunctionType.Identity,
                                         accum_out=acc[:, i, 0:1])
                    nc.scalar.activation(out=junk, in_=rng,
                                         func=mybir.ActivationFunctionType.Square,
                                         accum_out=acc[:, i, 1:2])
                nc.vector.bn_aggr(out=mva, in_=stats8a)
                nc.vector.tensor_copy(out=st[0][:, 0:1], in_=mva[:, 0:1])
                nc.vector.tensor_mul(out=st[0][:, 1:2], in0=mva[:, 0:1], in1=mva[:, 0:1])
                nc.vector.tensor_add(out=st[0][:, 1:2], in0=st[0][:, 1:2], in1=mva[:, 1:2])

                nc.vector.bn_aggr(out=mv, in_=stats2)
                n_dve = float(2 * 1024)
                n_tot = float(N)
                sums = small.tile([128, 2], F32, tag="sums", name="sums")
                nc.vector.tensor_tensor(out=sums, in0=acc[:, 0, :],
                                        in1=acc[:, 1, :], op=mybir.AluOpType.add)
                nc.vector.tensor_tensor(out=sums, in0=sums,
                                        in1=acc[:, 2, :], op=mybir.AluOpType.add)
                # E[x] = (mean_dve*n_dve + sum_act)/N
                nc.vector.scalar_tensor_tensor(out=st[1][:, 0:1], in0=mv[:, 0:1],
                                               scalar=n_dve, in1=sums[:, 0:1],
                                               op0=mybir.AluOpType.mult,
                                               op1=mybir.AluOpType.add)
                nc.vector.tensor_scalar_mul(out=st[1][:, 0:1], in0=st[1][:, 0:1],
                                            scalar1=1.0 / n_tot)
                # E[x^2] = ((var+mean^2)*n_dve + sumsq_act)/N
                nc.vector.tensor_mul(out=st[1][:, 1:2], in0=mv[:, 0:1], in1=mv[:, 0:1])
                nc.vector.tensor_add(out=st[1][:, 1:2], in0=st[1][:, 1:2], in1=mv[:, 1:2])
                nc.vector.scalar_tensor_tensor(out=st[1][:, 1:2], in0=st[1][:, 1:2],
                                               scalar=n_dve, in1=sums[:, 1:2],
                                               op0=mybir.AluOpType.mult,
                                               op1=mybir.AluOpType.add)
                nc.vector.tensor_scalar_mul(out=st[1][:, 1:2], in0=st[1][:, 1:2],
                                            scalar1=1.0 / n_tot)

                # M2 = (Wq^T Wk) tiles: m2_sb[cs][p, f] = Mk[f, cs*128+p]
                # (emitted after stats so the ACT/DVE evacs queue behind them)
                for cs in range(2):
                    m2_ps = pspre.tile([128, C], F32, tag="m2ps", name="m2ps", bufs=2)
                    for ot in range(2):
                        nc.tensor.matmul(m2_ps, lhsT=wq_nt[ot][:, cs * 128:(cs + 1) * 128],
                                         rhs=wk_nt[ot], start=(ot == 0), stop=(ot == 1))
                    if cs == 0:
                        nc.scalar.copy(out=m2_sb[cs], in_=m2_ps)
                    else:
                        nc.vector.tensor_copy(out=m2_sb[cs], in_=m2_ps)
                # w~ = Wk^T bq (per c-tile), later scaled by a into w2
                wt_ps2 = pspre.tile([128, 2], F32, tag="wtps", name="wtps")
                wt_ps = [wt_ps2[:, ct:ct + 1] for ct in range(2)]
                for ct in range(2):
                    for ot in range(2):
                        nc.tensor.matmul(wt_ps[ct],
                                         lhsT=wk_nt[ot][:, ct * 128:(ct + 1) * 128],
                                         rhs=bq_half(bq2, ot), start=(ot == 0), stop=(ot == 1))
                # bridge the PE across the stats window (HAM stays warm)
                for _ in range(12):
                    nc.tensor.matmul(wp_ps, lhsT=ident, rhs=warm_rhs, start=True, stop=True)

                grs_t = []
                for t in range(2):
                    # per-tile group stats -> rsqrt chain
                    ps_g = pspre.tile([GH, 2], F32, tag="gst", name="gst", bufs=2)
                    nc.tensor.matmul(ps_g, lhsT=ig_t[t],
                                     rhs=st[t], start=True, stop=True)
                    tg1 = small.tile([GH, 1], F32, tag=f"tg1{t}", name=f"tg1{t}")
                    tg2 = small.tile([GH, 1], F32, tag=f"tg2{t}", name=f"tg2{t}")
                    tg3 = small.tile([GH, 1], F32, tag=f"tg3{t}", name=f"tg3{t}")
                    tg4 = small.tile([GH, 1], F32, tag=f"tg4{t}", name=f"tg4{t}")
                    grs = small.tile([GH, 2], F32, tag=f"grs{t}", name=f"grs{t}")
                    nc.vector.tensor_copy(out=grs[:, 0:1], in_=ps_g[:, 0:1])
                    nc.vector.tensor_mul(out=tg1, in0=grs[:, 0:1], in1=grs[:, 0:1])
                    nc.vector.tensor_tensor(out=tg1, in0=ps_g[:, 1:2], in1=tg1,
                                            op=mybir.AluOpType.subtract)
                    nc.vector.tensor_scalar_add(out=tg1, in0=tg1, scalar1=EPS)
                    # rsqrt(v) on DVE: quake seed + 1 Newton step
                    nc.vector.tensor_scalar(out=tg2.bitcast(I32), in0=tg1.bitcast(I32),
                                            scalar1=1, scalar2=None,
                                            op0=mybir.AluOpType.logical_shift_right)
                    nc.vector.tensor_scalar(out=tg2.bitcast(I32), in0=tg2.bitcast(I32),
                                            scalar1=-1, scalar2=0x5f3759df,
                                            op0=mybir.AluOpType.mult,
                                            op1=mybir.AluOpType.add)
                    nc.vector.tensor_mul(out=tg3, in0=tg2, in1=tg2)
                    nc.vector.tensor_mul(out=tg3, in0=tg3, in1=tg1)
                    nc.vector.tensor_scalar(out=tg4, in0=tg3, scalar1=-0.5,
                                            scalar2=1.5, op0=mybir.AluOpType.mult,
                                            op1=mybir.AluOpType.add)
                    nc.vector.tensor_mul(out=tg2, in0=tg2, in1=tg4)
                    nc.vector.tensor_copy(out=grs[:, 1:2], in_=tg2)
                    grs_t.append(grs)

                mc2 = pspre.tile([128, 2, 2], F32, tag="mcrs", name="mcrs")
                for t in range(2):
                    nc.tensor.matmul(mc2[:, t, :], lhsT=igt_sb[t],
                                     rhs=grs_t[t], start=True, stop=True)
                nc.vector.tensor_mul(out=a2, in0=vcol2(VG), in1=mc2[:, :, 1])
                nc.vector.tensor_mul(out=b2, in0=mc2[:, :, 0], in1=a2)
                nc.vector.tensor_tensor(out=b2, in0=vcol2(VB),
                                        in1=b2, op=mybir.AluOpType.subtract)

                # fold GN scale into Wv rows (split ACT/DVE); w2 = a (.) Wk^T bq
                nc.scalar.activation(out=wv_pk[:, 0, :], in_=wraw_v[0],
                                     func=mybir.ActivationFunctionType.Identity,
                                     scale=a_t[0])
                nc.vector.tensor_scalar_mul(out=wv_pk[:, 1, :], in0=wraw_v[1],
                                            scalar1=a_t[1])
                nc.vector.tensor_mul(out=w22, in0=wt_ps2, in1=a2)
                # Hq = a (.) x + b from the fp8 query chunk (chunk 0 after rotation)
                nc.gpsimd.tensor_scalar(out=hq_bf[0], in0=x_pk[:, 0, 0, :],
                                        scalar1=a_t[0], scalar2=b_t[0],
                                        op0=mybir.AluOpType.mult,
                                        op1=mybir.AluOpType.add)
                nc.vector.tensor_scalar(out=hq_bf[1], in0=x_pk[:, 0, 1, :],
                                        scalar1=a_t[1], scalar2=b_t[1],
                                        op0=mybir.AluOpType.mult,
                                        op1=mybir.AluOpType.add)

            # ---- cv, wpcv, G'' ----
            with tc.tile_pool(name="psgen", bufs=1, space="PSUM") as psgen:
                for m in range(2):
                    cp = psgen.tile([128, 1], F32, tag="cps", name="cps", bufs=1)
                    for t in range(2):
                        nc.tensor.matmul(cp, lhsT=wraw_v[t][:, m * 128:(m + 1) * 128],
                                         rhs=b_t[t], start=(t == 0), stop=(t == 1))
                    nc.vector.tensor_tensor(out=cv[m], in0=cp,
                                            in1=vcol(VBV, m),
                                            op=mybir.AluOpType.add)
                    nc.vector.tensor_copy(out=cvbf[:, m:m + 1], in_=cv[m])
                # wpcv = Wp @ cv; bpv = bp + wpcv folds into the residual prep
                w_ps = psgen.tile([128, 2], F32, tag="wps", name="wps")
                for m in range(2):
                    for ct in range(2):
                        nc.tensor.matmul(w_ps[:, m:m + 1],
                                         lhsT=wpT_t[ct][:, m * 128:(m + 1) * 128],
                                         rhs=cvbf[:, ct:ct + 1],
                                         start=(ct == 0), stop=(ct == 1))
                nc.vector.tensor_tensor(out=bpv2, in0=w_ps, in1=vcol2(VBP),
                                        op=mybir.AluOpType.add)
                for ct in range(2):
                    g_ps = psgen.tile([128, NQ], F32, tag="gps", name="gps", bufs=2)
                    for qh in range(2):
                        for cs in range(2):
                            nc.tensor.matmul(g_ps[:, qh * 512:(qh + 1) * 512],
                                             lhsT=m2_sb[cs][:, ct * 128:(ct + 1) * 128],
                                             rhs=hq_bf[cs][:, qh * 512:(qh + 1) * 512],
                                             start=(cs == 0), stop=(cs == 1))
                    dst = g_pk[:, ct, :]
                    if ct == 0:
                        nc.scalar.activation(out=dst, in_=g_ps,
                                             func=mybir.ActivationFunctionType.Identity,
                                             bias=w2[ct], scale=a_t[ct])
                    else:
                        nc.vector.tensor_scalar(out=dst, in0=g_ps, scalar1=a_t[ct],
                                                scalar2=w2[ct],
                                                op0=mybir.AluOpType.mult,
                                                op1=mybir.AluOpType.add)

            def x_lhsT(kt):  # [128, 2, 128] channel-packed key-tile slice
                return x_pk[:, kt // 8, :, (kt % 8) * 128:(kt % 8 + 1) * 128]

            o_sb = [attn.tile([128, C], BF16, tag=f"o{jq}", name=f"o{jq}")
                    for jq in range(NQT)]
            ot_bf = attn.tile([128, 2, NQ], BF16, tag="otbf", name="otbf")
            y_sb = [attn.tile([128, NQ], BF16, tag=f"y{t}", name=f"y{t}") for t in range(2)]

            # ---- S^T -> exp (ACT|DVE) + V ----
            with (
                tc.tile_pool(name="pss", bufs=3, space="PSUM") as pss,
                tc.tile_pool(name="psv", bufs=2, space="PSUM") as psv,
            ):
                for j in range(NJ):
                    for s in range(2):
                        kt = 2 * j + s
                        sp = pss.tile([128, NQ], F32, tag="s", name="s")
                        for h in range(2):
                            nc.tensor.matmul(sp[:, h * 512:(h + 1) * 512],
                                             lhsT=x_lhsT(kt),
                                             rhs=g_pk[:, :, h * 512:(h + 1) * 512],
                                             start=True, stop=True,
                                             perf_mode=mybir.MatmulPerfMode.DoubleRow)
                        if s == 0:
                            nc.scalar.activation(out=pt[j][:, s, :], in_=sp, bias=ebias,
                                                 func=mybir.ActivationFunctionType.Exp,
                                                 scale=SCALE)
                        else:
                            nc.vector.tensor_scalar(
                                out=pt[j][:, s, :].bitcast(U8), in0=sp,
                                scalar1=float(EXP_A * SCALE),
                                scalar2=float(EXP_B + LOGIT_BIAS * EXP_A),
                                op0=mybir.AluOpType.mult, op1=mybir.AluOpType.add)
                    # V block j rides the S loop
                    vp = psv.tile([128, 2, C], F32, tag="vps", name="vp")
                    for s in range(2):
                        kt = 2 * j + s
                        nc.tensor.matmul(vp[:, s, :],
                                         lhsT=x_lhsT(kt),
                                         rhs=wv_pk,
                                         start=True, stop=True,
                                         perf_mode=mybir.MatmulPerfMode.DoubleRow)
                    if j in ACT_V_JS:
                        nc.scalar.copy(out=vt1[j][:, :, 0:C], in_=vp)
                    else:
                        nc.vector.tensor_copy(out=vt1[j][:, :, 0:C], in_=vp)

            # ---- O chains, PE transposes, proj, residual, store ----
            with (
                tc.tile_pool(name="ob2", bufs=4, space="PSUM") as ob2,
                tc.tile_pool(name="pst", bufs=2, space="PSUM") as pst,
                tc.tile_pool(name="psy", bufs=2, space="PSUM") as psy,
            ):
                def chain_mm(ob, jq, j):
                    nc.tensor.matmul(ob,
                                     lhsT=pt[j][:, :, jq * 128:(jq + 1) * 128],
                                     rhs=vt1[j][:, :, 0:C + 1],
                                     start=(j == 0), stop=(j == NJ - 1),
                                     perf_mode=mybir.MatmulPerfMode.DoubleRow)

                def evac_chain(ob, jq):
                    rec = small.tile([128, 1], F32, tag="rec2", name="rec2", bufs=4)
                    nc.vector.reciprocal(out=rec, in_=ob[:, C:C + 1])
                    nc.scalar.activation(out=o_sb[jq], in_=ob[:, 0:C],
                                         func=mybir.ActivationFunctionType.Identity,
                                         scale=rec)

                def transpose_block(jq):
                    for ct in range(2):
                        tp = pst.tile([128, 128], BF16, tag="tp", name="tp")
                        nc.tensor.transpose(tp, o_sb[jq][:, ct * 128:(ct + 1) * 128],
                                            ident)
                        dst = ot_bf[:, ct, jq * 128:(jq + 1) * 128]
                        if (jq + ct) % 2 == 0:
                            nc.scalar.copy(out=dst, in_=tp)
                        else:
                            nc.vector.tensor_copy(out=dst, in_=tp)

                def proj_block(n):
                    for m in range(2):
                        yp = psy.tile([128, 512], F32, tag="yps", name="yps")
                        for ct in range(2):
                            nc.tensor.matmul(yp,
                                             lhsT=wpT_t[ct][:, m * 128:(m + 1) * 128],
                                             rhs=ot_bf[:, ct, n * 512:(n + 1) * 512],
                                             start=(ct == 0), stop=(ct == 1))
                        # y = yp + (bp + Wp@cv) + xq  (residual bias folded here)
                        nc.vector.scalar_tensor_tensor(
                            out=y_sb[m][:, n * 512:(n + 1) * 512], in0=yp,
                            scalar=bpv2[:, m:m + 1],
                            in1=xq_f[m][:, n * 512:(n + 1) * 512],
                            op0=mybir.AluOpType.add, op1=mybir.AluOpType.add)
                        nc.sync.dma_start(out=y[m, n],
                                          in_=y_sb[m][:, n * 512:(n + 1) * 512])

                obs = {}

                def chains(pair):
                    for jq in pair:
                        obs[jq] = ob2.tile([128, C + 1], F32, tag="ob2", name="ob2")
                    for j in range(NJ):
                        for jq in pair:
                            chain_mm(obs[jq], jq, j)

                chains((0, 1))
                chains((2, 3))
                evac_chain(obs[0], 0)
                evac_chain(obs[1], 1)
                transpose_block(0)
                transpose_block(1)
                chains((4, 5))
                evac_chain(obs[2], 2)
                evac_chain(obs[3], 3)
                transpose_block(2)
                transpose_block(3)
                proj_block(0)
                chains((6, 7))
                evac_chain(obs[4], 4)
                evac_chain(obs[5], 5)
                transpose_block(4)
                transpose_block(5)
                evac_chain(obs[6], 6)
                evac_chain(obs[7], 7)
                transpose_block(6)
                transpose_block(7)
                proj_block(1)

    nc.compile()
    return nc


def bq_half(bq2, ot):
    return bq2[:, ot:ot + 1]


_NC_CACHE = None


def _get_nc():
    global _NC_CACHE
    if _NC_CACHE is None:
        _NC_CACHE = build_nc()
    return _NC_CACHE


def make_in_maps(inputs):
    x = np.ascontiguousarray(np.asarray(inputs["x"], np.float32))
    xf = x.reshape(B, C, N)
    xf_bf = xf.astype(ml_dtypes.float8_e4m3)
    group = np.arange(C) // (C // G)  # channel -> group
    ig = np.zeros((2, 128, GH), np.float32)
    igt = np.zeros((2, GH, 128), np.float32)
    for c in range(C):
        t = c // 128
        ig[t, c % 128, group[c] - t * GH] = 1.0 / (C // G)
        igt[t, group[c] - t * GH, c % 128] = 1.0
    vecs = np.zeros((2, 128, 5), np.float32)
    for t in range(2):
        sl = slice(t * 128, (t + 1) * 128)
        vecs[t, :, VG] = np.asarray(inputs["gn_gamma"])[sl]
        vecs[t, :, VB] = np.asarray(inputs["gn_beta"])[sl]
        vecs[t, :, VBQ] = np.asarray(inputs["bq"])[sl]
        vecs[t, :, VBV] = np.asarray(inputs["bv"])[sl]
        vecs[t, :, VBP] = np.asarray(inputs["bp"])[sl]
    # bf16 pack [128, 3, 2, C]: wq | wk | wpT, each [half, row-in-half, :]
    bfpk = np.empty((128, 3, 2, C), np.float32)
    bfpk[:, 0] = np.asarray(inputs["Wq"], np.float32).reshape(2, 128, C).transpose(1, 0, 2)
    bfpk[:, 1] = np.asarray(inputs["Wk"], np.float32).reshape(2, 128, C).transpose(1, 0, 2)
    bfpk[:, 2] = np.asarray(inputs["Wp"], np.float32).T.reshape(2, 128, C).transpose(1, 0, 2)
    # f32 pack [128, FPK_W]: wvT (t-major) | vecs (v-major) | ig
    fpk = np.empty((128, FPK_W), np.float32)
    fpk[:, 0:2 * C] = np.asarray(inputs["Wv"], np.float32).T.reshape(2, 128, C) \
        .transpose(1, 0, 2).reshape(128, 2 * C)
    fpk[:, VEC_OFF:VEC_OFF + 10] = vecs.transpose(1, 2, 0).reshape(128, 10)
    fpk[:, IG_OFF:] = ig.transpose(1, 0, 2).reshape(128, 2 * GH)
    common = {
        "bfpk": np.ascontiguousarray(bfpk.astype(ml_dtypes.bfloat16)),
        "fpk": np.ascontiguousarray(fpk),
        "igt": igt,
    }
    in_maps = []
    for core in range(8):
        b, ch = core // 4, core % 4
        xb_cm = xf_bf[b].reshape(2, 128, 4, 1024).transpose(2, 1, 0, 3).reshape(4, 128, 2048)
        # rotate so the core's query chunk sits at index 0 (key order is
        # permutation-invariant through S -> P -> V -> O)
        rot = [(ch + i) % 4 for i in range(4)]
        in_maps.append({
            "xb": np.ascontiguousarray(xb_cm[rot]),
            "xq": np.ascontiguousarray(xf[b][:, ch * NQ:(ch + 1) * NQ]),
            **common,
        })
    return in_maps, x


def run(inputs, trace=False, tmpdir=None):
    nc = _get_nc()
    in_maps, x = make_in_maps(inputs)
    res = run_bass_kernel_spmd(nc, in_maps, core_ids=list(range(8)),
                               trace=trace, tmpdir=tmpdir)
    out = np.empty((B, C, N), np.float32)
    for core in range(8):
        b, ch = core // 4, core % 4
        yc = np.asarray(res.results[core]["y"], np.float32)  # [2, 2, 128, 512]
        out[b][:, ch * NQ:(ch + 1) * NQ] = yc.transpose(0, 2, 1, 3).reshape(C, NQ)
    return out.reshape(B, C, 16, 16, 16), res


def kernel(**inputs) -> np.ndarray:
    out, _ = run(inputs, trace=False)
    return out
